# revision 9
# baseline (speedup 1.0000x reference)
"""Trainium2 Bass kernel for nn_Attention_21303037788751 (sparse_attention).

Reference computation (B=16, N=512, F=256, H=8, D=64):
    qkv  = node @ W_qkv                      -> q, k, v  [B,H,N,D]
    attn = softmax(q k^T / sqrt(D)) + 0.5*adj + 0.5*exp(-dist)
    out  = (attn @ v) reshaped  @ W_out + b_out

Sharding: data-parallel over batch, 2 batches per core on 8 NeuronCores.

Per-core program (col-major formulation; all matmuls float32r):
    nodeT = node^T (PE transpose)            [F, N]
    qT,kT per head-pair = W_qk^T @ nodeT     [128=(2 heads), N]
    v     = node @ W_v (row-major)           [N, H*D]
    ST_h  = kT_h-slices ^T @ qT_h            [N_j, N_i]  (K=64, head pairs on
                                              disjoint PE row strips)
    E_h   = exp(ST_h / 8)  (ACT, PSUM->SBUF) -- logits are tiny, no max pass
    OT1_h = vaug_h^T @ E_h: the stationary operand is V_h augmented with a
            ones column (odd heads also get 63 leading zero columns), so one
            accumulation group yields both V^T E and the softmax column sums,
            with odd heads landing at partitions 64..127.
    softmax part = OT1_h * bcast(1/sums)      (DVE recip + DMA bcast)
    G     = adj + exp(-dist); GT by PE transpose
    OT2_p = [v_e|v_o]^T @ GT                  [128, N_i]
    Y     = softmaxT^T @ W_out + OT2^T @ (0.5*W_out) + ones^T @ b_out
"""

import sys

sys.path.insert(0, "/opt/trn_rl_repo")

import numpy as np

B, N, F = 16, 512, 256
H, D = 8, 64
INNER = H * D          # 512
NC_COUNT = 8
PB = B // NC_COUNT     # batches per core
P = 128
SCALE = D ** -0.5      # 0.125
VBLK = 193             # per-pair vaug block: [1 | 0*63 | v_o(64) | v_e(64) | 1]

_CACHE = {}


def _col_perm():
    """Column permutation of W_qkv: head-pair [q_h0|q_h1|k_h0|k_h1] blocks,
    then all v columns grouped by head."""
    order = []
    for p in range(H // 2):
        h0, h1 = 2 * p, 2 * p + 1
        order += [h0 * 192 + d for d in range(64)]
        order += [h1 * 192 + d for d in range(64)]
        order += [h0 * 192 + 64 + d for d in range(64)]
        order += [h1 * 192 + 64 + d for d in range(64)]
    for h in range(H):
        order += [h * 192 + 128 + d for d in range(64)]
    return np.array(order)


def build_program():
    import concourse.bass as bass
    import concourse.tile as tile
    from concourse import bacc, mybir
    from concourse.masks import make_identity

    f32 = mybir.dt.float32
    f32r = mybir.dt.float32r

    nc = bacc.Bacc("TRN2", target_bir_lowering=False, debug=False,
                   num_devices=NC_COUNT)

    node_d = nc.dram_tensor("node", [PB, N, F], f32, kind="ExternalInput").ap()
    adj_d = nc.dram_tensor("adj", [PB, N, N], f32, kind="ExternalInput").ap()
    dist_d = nc.dram_tensor("dist", [PB, N, N], f32, kind="ExternalInput").ap()
    wqkv_d = nc.dram_tensor("wqkv", [F, 3 * INNER], f32, kind="ExternalInput").ap()
    wout_d = nc.dram_tensor("wout", [INNER, F], f32, kind="ExternalInput").ap()
    wouth_d = nc.dram_tensor("wouth", [INNER, F], f32, kind="ExternalInput").ap()
    bout_d = nc.dram_tensor("bout", [1, F], f32, kind="ExternalInput").ap()
    out_d = nc.dram_tensor("out", [PB, N, F], f32, kind="ExternalOutput").ap()

    with tile.TileContext(nc) as tc:
        with tc.tile_pool(name="const", bufs=1) as cpool, \
             tc.tile_pool(name="stage", bufs=1) as spool, \
             tc.tile_pool(name="work1", bufs=1) as wpool1, \
             tc.tile_pool(name="work2", bufs=2) as wpool, \
             tc.tile_pool(name="ps_st", bufs=1, space="PSUM") as ps_st, \
             tc.tile_pool(name="ps_ot1", bufs=2, space="PSUM") as ps_ot1, \
             tc.tile_pool(name="ps_misc", bufs=2, space="PSUM") as ps_misc:

            # ---- constants -------------------------------------------------
            ident = cpool.tile([P, P], f32)
            make_identity(nc, ident)

            wqkv_st = spool.tile([P, 2, 3 * INNER], f32, tag="wqkv_st")
            nc.sync.dma_start(wqkv_st[:], wqkv_d.rearrange("(kt p) m -> p kt m", p=P))
            wqkv_r = cpool.tile([P, 2, 3 * INNER], f32r)
            nc.vector.tensor_copy(wqkv_r[:], wqkv_st[:])

            wout_st = spool.tile([P, 4, F], f32, tag="wout_st")
            nc.sync.dma_start(wout_st[:], wout_d.rearrange("(kt p) f -> p kt f", p=P))
            wout_r = cpool.tile([P, 4, F], f32r)
            nc.vector.tensor_copy(wout_r[:], wout_st[:])
            wouth_st = spool.tile([P, 4, F], f32, tag="wouth_st")
            nc.sync.dma_start(wouth_st[:], wouth_d.rearrange("(kt p) f -> p kt f", p=P))
            wouth_r = cpool.tile([P, 4, F], f32r)
            nc.vector.tensor_copy(wouth_r[:], wouth_st[:])

            bout_st = spool.tile([1, F], f32, tag="bout_st")
            nc.sync.dma_start(bout_st[:], bout_d[:])
            bout_r = cpool.tile([1, F], f32r)
            nc.vector.tensor_copy(bout_r[:], bout_st[:])

            ones_st = cpool.tile([1, P], f32)
            nc.vector.memset(ones_st[:], 1.0)
            ones_row = cpool.tile([1, P], f32r)
            nc.vector.tensor_copy(ones_row[:], ones_st[:])

            # vaug pad pattern: block head [1, 0*63] (odd-head ones + zeros),
            # block tail col 192 is the even-head ones column
            pat_st = cpool.tile([P, 64], f32)
            nc.vector.memset(pat_st[:], 0.0)
            nc.vector.memset(pat_st[:, 0:1], 1.0)
            one_st = cpool.tile([P, 1], f32)
            nc.vector.memset(one_st[:], 1.0)

            for b in range(PB):
                # ---- stage inputs -----------------------------------------
                node_sb = wpool1.tile([P, 4, F], f32, tag="node")
                nc.sync.dma_start(node_sb[:],
                                  node_d[b].rearrange("(nb p) f -> p nb f", p=P))
                adj_sb = wpool.tile([P, 4, N], f32, tag="adj")
                nc.sync.dma_start(adj_sb[:],
                                  adj_d[b].rearrange("(ib p) j -> p ib j", p=P))
                dist_sb = wpool.tile([P, 4, N], f32, tag="dist")
                nc.sync.dma_start(dist_sb[:],
                                  dist_d[b].rearrange("(ib p) j -> p ib j", p=P))

                # ---- nodeT [F, N] ----------------------------------------
                nodeT_r = wpool1.tile([P, 2, N], f32r, tag="nodeT")
                for kt in range(2):
                    tr_ps = ps_misc.tile([P, N], f32, tag="misc")
                    for nb in range(4):
                        nc.tensor.transpose(
                            tr_ps[:, nb * P:(nb + 1) * P],
                            node_sb[:, nb, kt * P:(kt + 1) * P],
                            ident[:])
                    nc.vector.tensor_copy(nodeT_r[:, kt, :], tr_ps[:])

                # ---- G = adj + exp(-dist); GT -----------------------------
                expd_sb = wpool1.tile([P, 4, N], f32, tag="expd")
                nc.scalar.activation(expd_sb[:], dist_sb[:],
                                     mybir.ActivationFunctionType.Exp,
                                     scale=-1.0)
                g_sb = wpool1.tile([P, 4, N], f32, tag="g")
                nc.gpsimd.tensor_add(g_sb[:], adj_sb[:], expd_sb[:])
                gt_r = wpool.tile([P, 4, N], f32r, tag="gt")
                for jb in range(4):
                    tr_ps = ps_misc.tile([P, N], f32, tag="misc")
                    for ib in range(4):
                        nc.tensor.transpose(
                            tr_ps[:, ib * P:(ib + 1) * P],
                            g_sb[:, ib, jb * P:(jb + 1) * P],
                            ident[:])
                    nc.vector.tensor_copy(gt_r[:, jb, :], tr_ps[:])

                # ---- v projection into augmented stationary layout --------
                vaug = wpool.tile([P, 4, 4, VBLK], f32r, tag="v")
                nc.scalar.copy(
                    vaug[:, :, :, 0:64],
                    pat_st[:, None, None, :].to_broadcast((P, 4, 4, 64)))
                nc.scalar.copy(
                    vaug[:, :, :, 192:193],
                    one_st[:, None, None, :].to_broadcast((P, 4, 4, 1)))
                for jb in range(4):
                    v_ps = ps_misc.tile([P, N], f32, tag="misc")
                    for kt in range(2):
                        nc.tensor.matmul(
                            v_ps[:], nodeT_r[:, kt, jb * P:(jb + 1) * P],
                            wqkv_r[:, kt, 2 * INNER:3 * INNER],
                            start=(kt == 0), stop=(kt == 1))
                    v4 = v_ps[:].rearrange("q (pr two d) -> q pr two d",
                                           two=2, d=64)
                    nc.scalar.copy(vaug[:, jb, :, 128:192], v4[:, :, 0, :])
                    nc.scalar.copy(vaug[:, jb, :, 64:128], v4[:, :, 1, :])

                # ---- qT/kT projections ------------------------------------
                qq_r, kk_r = [], []
                for p in range(H // 2):
                    qq_ps = ps_misc.tile([P, N], f32, tag="misc")
                    kk_ps = ps_misc.tile([P, N], f32, tag="misc")
                    base = p * 256
                    for kt in range(2):
                        nc.tensor.matmul(
                            qq_ps[:], wqkv_r[:, kt, base:base + P],
                            nodeT_r[:, kt, :],
                            start=(kt == 0), stop=(kt == 1))
                    for kt in range(2):
                        nc.tensor.matmul(
                            kk_ps[:], wqkv_r[:, kt, base + P:base + 256],
                            nodeT_r[:, kt, :],
                            start=(kt == 0), stop=(kt == 1))
                    qq = wpool.tile([P, N], f32r, tag="qq")
                    kk = wpool.tile([P, N], f32r, tag="kk")
                    nc.vector.tensor_copy(qq[:], qq_ps[:])
                    nc.vector.tensor_copy(kk[:], kk_ps[:])
                    qq_r.append(qq)
                    kk_r.append(kk)

                # ---- attention per head -----------------------------------
                otfin_r = wpool.tile([P, 4, N], f32r, tag="otfin")
                ot2_r = wpool.tile([P, 4, N], f32r, tag="ot2")
                for h in range(H):
                    p, odd = h // 2, h % 2
                    lo = odd * 64              # row strip within the pair
                    st_ps = ps_st.tile([P, 4, N], f32, tag="st")
                    for jb in range(4):
                        nc.tensor.matmul(
                            st_ps[:, jb, :],
                            kk_r[p][lo:lo + 64, jb * P:(jb + 1) * P],
                            qq_r[p][lo:lo + 64, :],
                            start=True, stop=True)
                    expst = wpool.tile([P, 4, N], f32r, tag="expst")
                    nc.scalar.activation(expst[:], st_ps[:],
                                         mybir.ActivationFunctionType.Exp,
                                         scale=SCALE)

                    # V^T E and softmax column sums in one accumulation group
                    ot1_ps = ps_ot1.tile([P, N], f32, tag="ot1")
                    if not odd:
                        out_sl, av_sl, sm_sl = slice(0, 65), slice(0, 64), slice(64, 65)
                    else:
                        out_sl, av_sl, sm_sl = slice(0, P), slice(64, P), slice(0, 1)
                    for jb in range(4):
                        if not odd:
                            lhsT = vaug[:, jb, p, 128:VBLK]
                        else:
                            lhsT = vaug[:, jb, p, 0:128]
                        nc.tensor.matmul(
                            ot1_ps[out_sl, :], lhsT, expst[:, jb, :],
                            start=(jb == 0), stop=(jb == 3))

                    rec = wpool.tile([P, N], f32, tag="rec")
                    nc.vector.reciprocal(rec[sm_sl, :], ot1_ps[sm_sl, :])
                    recbc = wpool.tile([P, N], f32, tag="recbc")
                    nc.sync.dma_start(recbc[av_sl, :],
                                      rec[sm_sl, None, :].to_broadcast((1, 64, N)))
                    nc.vector.tensor_tensor(
                        otfin_r[av_sl, p, :], ot1_ps[av_sl, :], recbc[av_sl, :],
                        mybir.AluOpType.mult)

                # ---- OT2 per pair (G part) --------------------------------
                for p in range(H // 2):
                    ot2_ps = ps_misc.tile([P, N], f32, tag="misc")
                    for jb in range(4):
                        nc.tensor.matmul(
                            ot2_ps[:], vaug[:, jb, p, 64:192], gt_r[:, jb, :],
                            start=(jb == 0), stop=(jb == 3))
                    nc.scalar.copy(ot2_r[:, p, :], ot2_ps[:])

                # ---- output projection ------------------------------------
                for nb in range(4):
                    y_ps = ps_misc.tile([P, F], f32, tag="misc")
                    for kt in range(4):
                        nc.tensor.matmul(
                            y_ps[:], otfin_r[:, kt, nb * P:(nb + 1) * P],
                            wout_r[:, kt, :],
                            start=(kt == 0), stop=False)
                    for kt in range(4):
                        nc.tensor.matmul(
                            y_ps[:], ot2_r[:, kt, nb * P:(nb + 1) * P],
                            wouth_r[:, kt, :],
                            start=False, stop=False)
                    nc.tensor.matmul(y_ps[:], ones_row[:], bout_r[:],
                                     start=False, stop=True)
                    y_sb = wpool.tile([P, F], f32, tag="y")
                    nc.vector.tensor_copy(y_sb[:], y_ps[:])
                    nc.sync.dma_start(out_d[b, nb * P:(nb + 1) * P, :], y_sb[:])

    nc.compile()
    return nc


def _get_program():
    if "nc" not in _CACHE:
        _CACHE["nc"] = build_program()
    return _CACHE["nc"]


def run(inputs, trace=False):
    """Run on 8 cores; returns (full_output, BassKernelResults)."""
    from concourse.bass_utils import run_bass_kernel_spmd

    nc = _get_program()
    wqkv_p = np.ascontiguousarray(inputs["W_qkv"][:, _col_perm()], dtype=np.float32)
    wout = np.ascontiguousarray(inputs["W_out"], dtype=np.float32)
    swap = np.arange(INNER).reshape(4, 2, 64)[:, ::-1, :].reshape(-1)
    wouth = np.ascontiguousarray(0.5 * wout[swap], dtype=np.float32)
    bout = np.ascontiguousarray(inputs["b_out"], dtype=np.float32).reshape(1, F)

    in_maps = []
    for c in range(NC_COUNT):
        sl = slice(c * PB, (c + 1) * PB)
        in_maps.append({
            "node": np.ascontiguousarray(inputs["node"][sl], dtype=np.float32),
            "adj": np.ascontiguousarray(inputs["adj"][sl], dtype=np.float32),
            "dist": np.ascontiguousarray(inputs["dist"][sl], dtype=np.float32),
            "wqkv": wqkv_p,
            "wout": wout,
            "wouth": wouth,
            "bout": bout,
        })
    res = run_bass_kernel_spmd(nc, in_maps, core_ids=list(range(NC_COUNT)),
                               trace=trace)
    out = np.concatenate([res.results[c]["out"] for c in range(NC_COUNT)], axis=0)
    return out, res


def kernel(node, adj, dist, node_mask, adj_mask, dist_mask, W_qkv, W_out, b_out):
    inputs = {"node": np.asarray(node), "adj": np.asarray(adj),
              "dist": np.asarray(dist), "W_qkv": np.asarray(W_qkv),
              "W_out": np.asarray(W_out), "b_out": np.asarray(b_out)}
    out, _ = run(inputs, trace=False)
    return out


# revision 12
# speedup vs baseline: 1.2474x; 1.2474x over previous
"""Trainium2 Bass kernel for nn_Attention_21303037788751 (sparse_attention).

Reference computation (B=16, N=512, F=256, H=8, D=64):
    qkv  = node @ W_qkv                      -> q, k, v  [B,H,N,D]
    attn = softmax(q k^T / sqrt(D)) + 0.5*adj + 0.5*exp(-dist)
    out  = (attn @ v) reshaped  @ W_out + b_out

Sharding: data-parallel over batch, 2 batches per core on 8 NeuronCores.

Per-core program (col-major formulation; all matmuls float32r):
    nodeT = node^T (PE transpose)            [F, N]
    qT,kT per head-pair = W_qk^T @ nodeT     [128=(2 heads), N]
    v     = node @ W_v (row-major)           [N, H*D]
    ST_h  = kT_h-slices ^T @ qT_h            [N_j, N_i]  (K=64, head pairs on
                                              disjoint PE row strips)
    E_h   = exp(ST_h / 8)  (ACT, PSUM->SBUF) -- logits are tiny, no max pass
    OT1_h = vaug_h^T @ E_h: the stationary operand is V_h augmented with a
            ones column (odd heads also get 63 leading zero columns), so one
            accumulation group yields both V^T E and the softmax column sums,
            with odd heads landing at partitions 64..127.
    softmax part = OT1_h * bcast(1/sums)      (DVE recip + DMA bcast)
    G     = adj + exp(-dist); GT by PE transpose
    OT2_p = [v_e|v_o]^T @ GT                  [128, N_i]
    Y     = softmaxT^T @ W_out + OT2^T @ (0.5*W_out) + ones^T @ b_out
"""

import sys

sys.path.insert(0, "/opt/trn_rl_repo")

import numpy as np

B, N, F = 16, 512, 256
H, D = 8, 64
INNER = H * D          # 512
NC_COUNT = 8
PB = B // NC_COUNT     # batches per core
P = 128
SCALE = D ** -0.5      # 0.125
VBLK = 193             # per-pair vaug block: [1 | 0*63 | v_o(64) | v_e(64) | 1]

_CACHE = {}


def _col_perm():
    """Column permutation of W_qkv: head-pair [q_h0|q_h1|k_h0|k_h1] blocks,
    then all v columns grouped by head."""
    order = []
    for p in range(H // 2):
        h0, h1 = 2 * p, 2 * p + 1
        order += [h0 * 192 + d for d in range(64)]
        order += [h1 * 192 + d for d in range(64)]
        order += [h0 * 192 + 64 + d for d in range(64)]
        order += [h1 * 192 + 64 + d for d in range(64)]
    for h in range(H):
        order += [h * 192 + 128 + d for d in range(64)]
    return np.array(order)


def build_program():
    import concourse.bass as bass
    import concourse.tile as tile
    from concourse import bacc, mybir
    from concourse.masks import make_identity

    f32 = mybir.dt.float32
    f32r = mybir.dt.float32r

    nc = bacc.Bacc("TRN2", target_bir_lowering=False, debug=False,
                   num_devices=NC_COUNT)

    node_d = nc.dram_tensor("node", [PB, N, F], f32, kind="ExternalInput").ap()
    adj_d = nc.dram_tensor("adj", [PB, N, N], f32, kind="ExternalInput").ap()
    dist_d = nc.dram_tensor("dist", [PB, N, N], f32, kind="ExternalInput").ap()
    wqkv_d = nc.dram_tensor("wqkv", [F, 3 * INNER], f32, kind="ExternalInput").ap()
    wout_d = nc.dram_tensor("wout", [INNER, F], f32, kind="ExternalInput").ap()
    wouth_d = nc.dram_tensor("wouth", [INNER, F], f32, kind="ExternalInput").ap()
    bout_d = nc.dram_tensor("bout", [1, F], f32, kind="ExternalInput").ap()
    out_d = nc.dram_tensor("out", [PB, N, F], f32, kind="ExternalOutput").ap()

    with tile.TileContext(nc) as tc:
        with tc.tile_pool(name="const", bufs=1) as cpool, \
             tc.tile_pool(name="stage", bufs=1) as spool, \
             tc.tile_pool(name="work1", bufs=1) as wpool1, \
             tc.tile_pool(name="work2", bufs=2) as wpool, \
             tc.tile_pool(name="qk", bufs=4) as qkpool, \
             tc.tile_pool(name="ps_st", bufs=2, space="PSUM") as ps_st, \
             tc.tile_pool(name="ps_ot1", bufs=2, space="PSUM") as ps_ot1, \
             tc.tile_pool(name="ps_misc", bufs=2, space="PSUM") as ps_misc:

            # ---- constants -------------------------------------------------
            ident = cpool.tile([P, P], f32)
            make_identity(nc, ident)

            wqkv_st = spool.tile([P, 2, 3 * INNER], f32, tag="stg")
            nc.sync.dma_start(wqkv_st[:], wqkv_d.rearrange("(kt p) m -> p kt m", p=P))
            wqkv_r = cpool.tile([P, 2, 3 * INNER], f32r)
            nc.vector.tensor_copy(wqkv_r[:], wqkv_st[:])

            wout_st = spool.tile([P, 4, F], f32, tag="stg")
            nc.sync.dma_start(wout_st[:], wout_d.rearrange("(kt p) f -> p kt f", p=P))
            wout_r = cpool.tile([P, 4, F], f32r)
            nc.vector.tensor_copy(wout_r[:], wout_st[:])
            wouth_st = spool.tile([P, 4, F], f32, tag="stg")
            nc.sync.dma_start(wouth_st[:], wouth_d.rearrange("(kt p) f -> p kt f", p=P))
            wouth_r = cpool.tile([P, 4, F], f32r)
            nc.vector.tensor_copy(wouth_r[:], wouth_st[:])

            bout_st = spool.tile([1, F], f32, tag="stg2")
            nc.sync.dma_start(bout_st[:], bout_d[:])
            bout_r = cpool.tile([1, F], f32r)
            nc.vector.tensor_copy(bout_r[:], bout_st[:])

            ones_st = cpool.tile([1, P], f32)
            nc.vector.memset(ones_st[:], 1.0)
            ones_row = cpool.tile([1, P], f32r)
            nc.vector.tensor_copy(ones_row[:], ones_st[:])

            # vaug pad pattern: block head [1, 0*63] (odd-head ones + zeros),
            # block tail col 192 is the even-head ones column
            pat_st = cpool.tile([P, 64], f32)
            nc.vector.memset(pat_st[:], 0.0)
            nc.vector.memset(pat_st[:, 0:1], 1.0)
            one_st = cpool.tile([P, 1], f32)
            nc.vector.memset(one_st[:], 1.0)

            for b in range(PB):
                # ---- stage inputs -----------------------------------------
                node_sb = wpool1.tile([P, 4, F], f32, tag="node")
                nc.sync.dma_start(node_sb[:],
                                  node_d[b].rearrange("(nb p) f -> p nb f", p=P))
                adj_sb = wpool.tile([P, 4, N], f32, tag="adj")
                nc.sync.dma_start(adj_sb[:],
                                  adj_d[b].rearrange("(ib p) j -> p ib j", p=P))
                dist_sb = wpool1.tile([P, 4, N], f32, tag="dist")
                nc.sync.dma_start(dist_sb[:],
                                  dist_d[b].rearrange("(ib p) j -> p ib j", p=P))

                # ---- nodeT [F, N] ----------------------------------------
                nodeT_r = wpool1.tile([P, 2, N], f32r, tag="nodeT")
                for kt in range(2):
                    tr_ps = ps_misc.tile([P, N], f32, tag="misc")
                    for nb in range(4):
                        nc.tensor.transpose(
                            tr_ps[:, nb * P:(nb + 1) * P],
                            node_sb[:, nb, kt * P:(kt + 1) * P],
                            ident[:])
                    nc.vector.tensor_copy(nodeT_r[:, kt, :], tr_ps[:])

                # ---- G = adj + exp(-dist); GT -----------------------------
                nc.scalar.activation(dist_sb[:], dist_sb[:],
                                     mybir.ActivationFunctionType.Exp,
                                     scale=-1.0)
                g_sb = adj_sb
                nc.gpsimd.tensor_add(g_sb[:], adj_sb[:], dist_sb[:])
                gt_r = wpool.tile([P, 4, N], f32r, tag="gt")
                for jb in range(4):
                    tr_ps = ps_misc.tile([P, N], f32, tag="misc")
                    for ib in range(4):
                        nc.tensor.transpose(
                            tr_ps[:, ib * P:(ib + 1) * P],
                            g_sb[:, ib, jb * P:(jb + 1) * P],
                            ident[:])
                    nc.vector.tensor_copy(gt_r[:, jb, :], tr_ps[:])

                # ---- v projection into augmented stationary layout --------
                vaug = wpool1.tile([P, 4, 4, VBLK], f32r, tag="v")
                nc.scalar.copy(
                    vaug[:, :, :, 0:64],
                    pat_st[:, None, None, :].to_broadcast((P, 4, 4, 64)))
                nc.scalar.copy(
                    vaug[:, :, :, 192:193],
                    one_st[:, None, None, :].to_broadcast((P, 4, 4, 1)))
                for jb in range(4):
                    v_ps = ps_misc.tile([P, N], f32, tag="misc")
                    for kt in range(2):
                        nc.tensor.matmul(
                            v_ps[:], nodeT_r[:, kt, jb * P:(jb + 1) * P],
                            wqkv_r[:, kt, 2 * INNER:3 * INNER],
                            start=(kt == 0), stop=(kt == 1))
                    v4 = v_ps[:].rearrange("q (pr two d) -> q pr two d",
                                           two=2, d=64)
                    nc.scalar.copy(vaug[:, jb, :, 128:192], v4[:, :, 0, :])
                    nc.scalar.copy(vaug[:, jb, :, 64:128], v4[:, :, 1, :])

                # ---- qT/kT projections ------------------------------------
                qq_r, kk_r = [], []
                for p in range(H // 2):
                    qq_ps = ps_misc.tile([P, N], f32, tag="misc")
                    kk_ps = ps_misc.tile([P, N], f32, tag="misc")
                    base = p * 256
                    for kt in range(2):
                        nc.tensor.matmul(
                            qq_ps[:], wqkv_r[:, kt, base:base + P],
                            nodeT_r[:, kt, :],
                            start=(kt == 0), stop=(kt == 1))
                    for kt in range(2):
                        nc.tensor.matmul(
                            kk_ps[:], wqkv_r[:, kt, base + P:base + 256],
                            nodeT_r[:, kt, :],
                            start=(kt == 0), stop=(kt == 1))
                    qq = qkpool.tile([P, N], f32r, tag="qq")
                    kk = qkpool.tile([P, N], f32r, tag="kk")
                    nc.vector.tensor_copy(qq[:], qq_ps[:])
                    nc.vector.tensor_copy(kk[:], kk_ps[:])
                    qq_r.append(qq)
                    kk_r.append(kk)

                # ---- attention per head -----------------------------------
                otun_r = wpool1.tile([P, 4, N], f32, tag="otun")
                otfin_r = wpool.tile([P, 4, N], f32r, tag="otfin")
                ot2_r = wpool.tile([P, 4, N], f32r, tag="ot2")
                recs = wpool1.tile([P, 4, N], f32, tag="recs")
                for h in range(H):
                    p, odd = h // 2, h % 2
                    lo = odd * 64              # row strip within the pair
                    expst = wpool.tile([P, 4, N], f32r, tag="expst")
                    for half in range(2):
                        st_ps = ps_st.tile([P, 2, N], f32, tag="st")
                        for j in range(2):
                            jb = half * 2 + j
                            nc.tensor.matmul(
                                st_ps[:, j, :],
                                kk_r[p][lo:lo + 64, jb * P:(jb + 1) * P],
                                qq_r[p][lo:lo + 64, :],
                                start=True, stop=True)
                        nc.scalar.activation(
                            expst[:, half * 2:half * 2 + 2, :], st_ps[:],
                            mybir.ActivationFunctionType.Exp, scale=SCALE)

                    # V^T E and softmax column sums in one accumulation group
                    ot1_ps = ps_ot1.tile([P, N], f32, tag="ot1")
                    if not odd:
                        out_sl, av_sl, sm_sl = slice(0, 65), slice(0, 64), slice(64, 65)
                    else:
                        out_sl, av_sl, sm_sl = slice(0, P), slice(64, P), slice(0, 1)
                    for jb in range(4):
                        if not odd:
                            lhsT = vaug[:, jb, p, 128:VBLK]
                        else:
                            lhsT = vaug[:, jb, p, 0:128]
                        nc.tensor.matmul(
                            ot1_ps[out_sl, :], lhsT, expst[:, jb, :],
                            start=(jb == 0), stop=(jb == 3))

                    nc.vector.tensor_copy(otun_r[av_sl, p, :], ot1_ps[av_sl, :])
                    nc.vector.reciprocal(recs[sm_sl, p, :], ot1_ps[sm_sl, :])

                # ---- batched softmax normalization ------------------------
                recbc = wpool1.tile([P, 4, N], f32, tag="recbc")
                for h in range(H):
                    p, odd = h // 2, h % 2
                    lo = odd * 64
                    sm = 64 - lo
                    nc.sync.dma_start(
                        recbc[lo:lo + 64, p, :],
                        recs[sm:sm + 1, p, None, :].to_broadcast((1, 64, N)))
                for p in range(H // 2):
                    nc.vector.tensor_mul(otfin_r[:, p, :], otun_r[:, p, :],
                                         recbc[:, p, :])

                # ---- OT2 per pair (G part) --------------------------------
                for p in range(H // 2):
                    ot2_ps = ps_misc.tile([P, N], f32, tag="misc")
                    for jb in range(4):
                        nc.tensor.matmul(
                            ot2_ps[:], vaug[:, jb, p, 64:192], gt_r[:, jb, :],
                            start=(jb == 0), stop=(jb == 3))
                    nc.scalar.copy(ot2_r[:, p, :], ot2_ps[:])

                # ---- output projection ------------------------------------
                for nb in range(4):
                    y_ps = ps_misc.tile([P, F], f32, tag="misc")
                    for kt in range(4):
                        nc.tensor.matmul(
                            y_ps[:], otfin_r[:, kt, nb * P:(nb + 1) * P],
                            wout_r[:, kt, :],
                            start=(kt == 0), stop=False)
                    for kt in range(4):
                        nc.tensor.matmul(
                            y_ps[:], ot2_r[:, kt, nb * P:(nb + 1) * P],
                            wouth_r[:, kt, :],
                            start=False, stop=False)
                    nc.tensor.matmul(y_ps[:], ones_row[:], bout_r[:],
                                     start=False, stop=True)
                    y_sb = wpool.tile([P, F], f32, tag="y")
                    nc.vector.tensor_copy(y_sb[:], y_ps[:])
                    nc.sync.dma_start(out_d[b, nb * P:(nb + 1) * P, :], y_sb[:])

    nc.compile()
    return nc


def _get_program():
    if "nc" not in _CACHE:
        _CACHE["nc"] = build_program()
    return _CACHE["nc"]


def run(inputs, trace=False):
    """Run on 8 cores; returns (full_output, BassKernelResults)."""
    from concourse.bass_utils import run_bass_kernel_spmd

    nc = _get_program()
    wqkv_p = np.ascontiguousarray(inputs["W_qkv"][:, _col_perm()], dtype=np.float32)
    wout = np.ascontiguousarray(inputs["W_out"], dtype=np.float32)
    swap = np.arange(INNER).reshape(4, 2, 64)[:, ::-1, :].reshape(-1)
    wouth = np.ascontiguousarray(0.5 * wout[swap], dtype=np.float32)
    bout = np.ascontiguousarray(inputs["b_out"], dtype=np.float32).reshape(1, F)

    in_maps = []
    for c in range(NC_COUNT):
        sl = slice(c * PB, (c + 1) * PB)
        in_maps.append({
            "node": np.ascontiguousarray(inputs["node"][sl], dtype=np.float32),
            "adj": np.ascontiguousarray(inputs["adj"][sl], dtype=np.float32),
            "dist": np.ascontiguousarray(inputs["dist"][sl], dtype=np.float32),
            "wqkv": wqkv_p,
            "wout": wout,
            "wouth": wouth,
            "bout": bout,
        })
    res = run_bass_kernel_spmd(nc, in_maps, core_ids=list(range(NC_COUNT)),
                               trace=trace)
    out = np.concatenate([res.results[c]["out"] for c in range(NC_COUNT)], axis=0)
    return out, res


def kernel(node, adj, dist, node_mask, adj_mask, dist_mask, W_qkv, W_out, b_out):
    inputs = {"node": np.asarray(node), "adj": np.asarray(adj),
              "dist": np.asarray(dist), "W_qkv": np.asarray(W_qkv),
              "W_out": np.asarray(W_out), "b_out": np.asarray(b_out)}
    out, _ = run(inputs, trace=False)
    return out


# revision 13
# speedup vs baseline: 1.2963x; 1.0392x over previous
"""Trainium2 Bass kernel for nn_Attention_21303037788751 (sparse_attention).

Reference computation (B=16, N=512, F=256, H=8, D=64):
    qkv  = node @ W_qkv                      -> q, k, v  [B,H,N,D]
    attn = softmax(q k^T / sqrt(D)) + 0.5*adj + 0.5*exp(-dist)
    out  = (attn @ v) reshaped  @ W_out + b_out

Sharding: data-parallel over batch, 2 batches per core on 8 NeuronCores.

Per-core program (col-major formulation; all matmuls float32r):
    nodeT = node^T (PE transpose)            [F, N]
    qT,kT per head-pair = W_qk^T @ nodeT     [128=(2 heads), N]
    v     = node @ W_v (row-major)           [N, H*D]
    ST_h  = kT_h-slices ^T @ qT_h            [N_j, N_i]  (K=64, head pairs on
                                              disjoint PE row strips)
    E_h   = exp(ST_h / 8)  (ACT, PSUM->SBUF) -- logits are tiny, no max pass
    OT1_h = vaug_h^T @ E_h: the stationary operand is V_h augmented with a
            ones column (odd heads also get 63 leading zero columns), so one
            accumulation group yields both V^T E and the softmax column sums,
            with odd heads landing at partitions 64..127.
    softmax part = OT1_h * bcast(1/sums)      (DVE recip + DMA bcast)
    G     = adj + exp(-dist); GT by PE transpose
    OT2_p = [v_e|v_o]^T @ GT                  [128, N_i]
    Y     = softmaxT^T @ W_out + OT2^T @ (0.5*W_out) + ones^T @ b_out
"""

import sys

sys.path.insert(0, "/opt/trn_rl_repo")

import numpy as np

B, N, F = 16, 512, 256
H, D = 8, 64
INNER = H * D          # 512
NC_COUNT = 8
PB = B // NC_COUNT     # batches per core
P = 128
SCALE = D ** -0.5      # 0.125
VBLK = 193             # per-pair vaug block: [1 | 0*63 | v_o(64) | v_e(64) | 1]

_CACHE = {}


def _col_perm():
    """Column permutation of W_qkv: head-pair [q_h0|q_h1|k_h0|k_h1] blocks,
    then all v columns grouped by head."""
    order = []
    for p in range(H // 2):
        h0, h1 = 2 * p, 2 * p + 1
        order += [h0 * 192 + d for d in range(64)]
        order += [h1 * 192 + d for d in range(64)]
        order += [h0 * 192 + 64 + d for d in range(64)]
        order += [h1 * 192 + 64 + d for d in range(64)]
    for h in range(H):
        order += [h * 192 + 128 + d for d in range(64)]
    return np.array(order)


def build_program():
    import concourse.bass as bass
    import concourse.tile as tile
    from concourse import bacc, mybir
    from concourse.masks import make_identity

    f32 = mybir.dt.float32
    f32r = mybir.dt.float32r

    nc = bacc.Bacc("TRN2", target_bir_lowering=False, debug=False,
                   num_devices=NC_COUNT)

    node_d = nc.dram_tensor("node", [PB, N, F], f32, kind="ExternalInput").ap()
    adj_d = nc.dram_tensor("adj", [PB, N, N], f32, kind="ExternalInput").ap()
    dist_d = nc.dram_tensor("dist", [PB, N, N], f32, kind="ExternalInput").ap()
    wqkv_d = nc.dram_tensor("wqkv", [F, 3 * INNER], f32, kind="ExternalInput").ap()
    wout_d = nc.dram_tensor("wout", [INNER, F], f32, kind="ExternalInput").ap()
    wouth_d = nc.dram_tensor("wouth", [INNER, F], f32, kind="ExternalInput").ap()
    bout_d = nc.dram_tensor("bout", [1, F], f32, kind="ExternalInput").ap()
    out_d = nc.dram_tensor("out", [PB, N, F], f32, kind="ExternalOutput").ap()

    with tile.TileContext(nc) as tc:
        with tc.tile_pool(name="const", bufs=1) as cpool, \
             tc.tile_pool(name="stage", bufs=1) as spool, \
             tc.tile_pool(name="work1", bufs=1) as wpool1, \
             tc.tile_pool(name="work2", bufs=2) as wpool, \
             tc.tile_pool(name="qk", bufs=4) as qkpool, \
             tc.tile_pool(name="ps_st", bufs=2, space="PSUM") as ps_st, \
             tc.tile_pool(name="ps_ot1", bufs=2, space="PSUM") as ps_ot1, \
             tc.tile_pool(name="ps_misc", bufs=2, space="PSUM") as ps_misc:

            # ---- constants -------------------------------------------------
            ident = cpool.tile([P, P], f32)
            make_identity(nc, ident)

            wqkv_st = spool.tile([P, 2, 3 * INNER], f32, tag="stg")
            nc.sync.dma_start(wqkv_st[:], wqkv_d.rearrange("(kt p) m -> p kt m", p=P))
            wqkv_r = cpool.tile([P, 2, 3 * INNER], f32r)
            nc.vector.tensor_copy(wqkv_r[:], wqkv_st[:])

            wout_st = spool.tile([P, 4, F], f32, tag="stg")
            nc.sync.dma_start(wout_st[:], wout_d.rearrange("(kt p) f -> p kt f", p=P))
            wout_r = cpool.tile([P, 4, F], f32r)
            nc.vector.tensor_copy(wout_r[:], wout_st[:])
            wouth_st = spool.tile([P, 4, F], f32, tag="stg")
            nc.sync.dma_start(wouth_st[:], wouth_d.rearrange("(kt p) f -> p kt f", p=P))
            wouth_r = cpool.tile([P, 4, F], f32r)
            nc.vector.tensor_copy(wouth_r[:], wouth_st[:])

            bout_st = spool.tile([1, F], f32, tag="stg2")
            nc.sync.dma_start(bout_st[:], bout_d[:])
            bout_r = cpool.tile([1, F], f32r)
            nc.vector.tensor_copy(bout_r[:], bout_st[:])

            ones_st = cpool.tile([1, P], f32)
            nc.vector.memset(ones_st[:], 1.0)
            ones_row = cpool.tile([1, P], f32r)
            nc.vector.tensor_copy(ones_row[:], ones_st[:])

            # vaug pad pattern: block head [1, 0*63] (odd-head ones + zeros),
            # block tail col 192 is the even-head ones column
            pat_st = cpool.tile([P, 64], f32)
            nc.vector.memset(pat_st[:], 0.0)
            nc.vector.memset(pat_st[:, 0:1], 1.0)
            one_st = cpool.tile([P, 1], f32)
            nc.vector.memset(one_st[:], 1.0)

            for b in range(PB):
                # ---- stage inputs -----------------------------------------
                node_sb = wpool1.tile([P, 4, F], f32, tag="node")
                nc.sync.dma_start(node_sb[:],
                                  node_d[b].rearrange("(nb p) f -> p nb f", p=P))
                adj_sb = wpool.tile([P, 4, N], f32, tag="adj")
                nc.sync.dma_start(adj_sb[:],
                                  adj_d[b].rearrange("(ib p) j -> p ib j", p=P))
                dist_sb = wpool1.tile([P, 4, N], f32, tag="dist")
                nc.sync.dma_start(dist_sb[:],
                                  dist_d[b].rearrange("(ib p) j -> p ib j", p=P))

                # ---- nodeT [F, N] ----------------------------------------
                nodeT_r = wpool1.tile([P, 2, N], f32r, tag="nodeT")
                for kt in range(2):
                    tr_ps = ps_misc.tile([P, N], f32, tag="misc")
                    for nb in range(4):
                        nc.tensor.transpose(
                            tr_ps[:, nb * P:(nb + 1) * P],
                            node_sb[:, nb, kt * P:(kt + 1) * P],
                            ident[:])
                    nc.vector.tensor_copy(nodeT_r[:, kt, :], tr_ps[:])

                # ---- v projection into augmented stationary layout --------
                vaug = wpool1.tile([P, 4, 4, VBLK], f32r, tag="v")
                nc.scalar.copy(
                    vaug[:, :, :, 0:64],
                    pat_st[:, None, None, :].to_broadcast((P, 4, 4, 64)))
                nc.scalar.copy(
                    vaug[:, :, :, 192:193],
                    one_st[:, None, None, :].to_broadcast((P, 4, 4, 1)))
                for jb in range(4):
                    v_ps = ps_misc.tile([P, N], f32, tag="misc")
                    for kt in range(2):
                        nc.tensor.matmul(
                            v_ps[:], nodeT_r[:, kt, jb * P:(jb + 1) * P],
                            wqkv_r[:, kt, 2 * INNER:3 * INNER],
                            start=(kt == 0), stop=(kt == 1))
                    v4 = v_ps[:].rearrange("q (pr two d) -> q pr two d",
                                           two=2, d=64)
                    nc.scalar.copy(vaug[:, jb, :, 128:192], v4[:, :, 0, :])
                    nc.scalar.copy(vaug[:, jb, :, 64:128], v4[:, :, 1, :])

                # ---- qT/kT projections ------------------------------------
                qq_r, kk_r = [], []
                for p in range(H // 2):
                    qq_ps = ps_misc.tile([P, N], f32, tag="misc")
                    kk_ps = ps_misc.tile([P, N], f32, tag="misc")
                    base = p * 256
                    for kt in range(2):
                        nc.tensor.matmul(
                            qq_ps[:], wqkv_r[:, kt, base:base + P],
                            nodeT_r[:, kt, :],
                            start=(kt == 0), stop=(kt == 1))
                    for kt in range(2):
                        nc.tensor.matmul(
                            kk_ps[:], wqkv_r[:, kt, base + P:base + 256],
                            nodeT_r[:, kt, :],
                            start=(kt == 0), stop=(kt == 1))
                    qq = qkpool.tile([P, N], f32r, tag="qq")
                    kk = qkpool.tile([P, N], f32r, tag="kk")
                    nc.vector.tensor_copy(qq[:], qq_ps[:])
                    nc.vector.tensor_copy(kk[:], kk_ps[:])
                    qq_r.append(qq)
                    kk_r.append(kk)

                # ---- G = adj + exp(-dist); GT -----------------------------
                nc.scalar.activation(dist_sb[:], dist_sb[:],
                                     mybir.ActivationFunctionType.Exp,
                                     scale=-1.0)
                g_sb = adj_sb
                nc.gpsimd.tensor_add(g_sb[:], adj_sb[:], dist_sb[:])
                gt_r = wpool.tile([P, 4, N], f32r, tag="gt")
                for jb in range(4):
                    tr_ps = ps_misc.tile([P, N], f32, tag="misc")
                    for ib in range(4):
                        nc.tensor.transpose(
                            tr_ps[:, ib * P:(ib + 1) * P],
                            g_sb[:, ib, jb * P:(jb + 1) * P],
                            ident[:])
                    nc.vector.tensor_copy(gt_r[:, jb, :], tr_ps[:])

                # ---- attention per head -----------------------------------
                otfin_r = wpool.tile([P, 4, N], f32r, tag="otfin")
                ot2_r = wpool.tile([P, 4, N], f32r, tag="ot2")
                for h in range(H):
                    p, odd = h // 2, h % 2
                    lo = odd * 64              # row strip within the pair
                    expst = wpool.tile([P, 4, N], f32r, tag="expst")
                    for half in range(2):
                        st_ps = ps_st.tile([P, 2, N], f32, tag="st")
                        for j in range(2):
                            jb = half * 2 + j
                            nc.tensor.matmul(
                                st_ps[:, j, :],
                                kk_r[p][lo:lo + 64, jb * P:(jb + 1) * P],
                                qq_r[p][lo:lo + 64, :],
                                start=True, stop=True)
                        nc.scalar.activation(
                            expst[:, half * 2:half * 2 + 2, :], st_ps[:],
                            mybir.ActivationFunctionType.Exp, scale=SCALE)

                    # V^T E and softmax column sums in one accumulation group
                    ot1_ps = ps_ot1.tile([P, N], f32, tag="ot1")
                    if not odd:
                        out_sl, av_sl, sm_sl = slice(0, 65), slice(0, 64), slice(64, 65)
                    else:
                        out_sl, av_sl, sm_sl = slice(0, P), slice(64, P), slice(0, 1)
                    for jb in range(4):
                        if not odd:
                            lhsT = vaug[:, jb, p, 128:VBLK]
                        else:
                            lhsT = vaug[:, jb, p, 0:128]
                        nc.tensor.matmul(
                            ot1_ps[out_sl, :], lhsT, expst[:, jb, :],
                            start=(jb == 0), stop=(jb == 3))

                    rec = wpool.tile([P, N], f32, tag="rec")
                    nc.vector.reciprocal(rec[sm_sl, :], ot1_ps[sm_sl, :])
                    recbc = wpool.tile([P, N], f32, tag="recbc")
                    nc.sync.dma_start(
                        recbc[av_sl, :],
                        rec[sm_sl, None, :].to_broadcast((1, 64, N)))
                    nc.vector.tensor_tensor(
                        otfin_r[av_sl, p, :], ot1_ps[av_sl, :], recbc[av_sl, :],
                        mybir.AluOpType.mult)

                # ---- OT2 per pair (G part) --------------------------------
                for p in range(H // 2):
                    ot2_ps = ps_misc.tile([P, N], f32, tag="misc")
                    for jb in range(4):
                        nc.tensor.matmul(
                            ot2_ps[:], vaug[:, jb, p, 64:192], gt_r[:, jb, :],
                            start=(jb == 0), stop=(jb == 3))
                    nc.scalar.copy(ot2_r[:, p, :], ot2_ps[:])

                # ---- output projection ------------------------------------
                for nb in range(4):
                    y_ps = ps_misc.tile([P, F], f32, tag="misc")
                    for kt in range(4):
                        nc.tensor.matmul(
                            y_ps[:], otfin_r[:, kt, nb * P:(nb + 1) * P],
                            wout_r[:, kt, :],
                            start=(kt == 0), stop=False)
                    for kt in range(4):
                        nc.tensor.matmul(
                            y_ps[:], ot2_r[:, kt, nb * P:(nb + 1) * P],
                            wouth_r[:, kt, :],
                            start=False, stop=False)
                    nc.tensor.matmul(y_ps[:], ones_row[:], bout_r[:],
                                     start=False, stop=True)
                    y_sb = wpool.tile([P, F], f32, tag="y")
                    nc.vector.tensor_copy(y_sb[:], y_ps[:])
                    nc.sync.dma_start(out_d[b, nb * P:(nb + 1) * P, :], y_sb[:])

    nc.compile()
    return nc


def _get_program():
    if "nc" not in _CACHE:
        _CACHE["nc"] = build_program()
    return _CACHE["nc"]


def run(inputs, trace=False):
    """Run on 8 cores; returns (full_output, BassKernelResults)."""
    from concourse.bass_utils import run_bass_kernel_spmd

    nc = _get_program()
    wqkv_p = np.ascontiguousarray(inputs["W_qkv"][:, _col_perm()], dtype=np.float32)
    wout = np.ascontiguousarray(inputs["W_out"], dtype=np.float32)
    swap = np.arange(INNER).reshape(4, 2, 64)[:, ::-1, :].reshape(-1)
    wouth = np.ascontiguousarray(0.5 * wout[swap], dtype=np.float32)
    bout = np.ascontiguousarray(inputs["b_out"], dtype=np.float32).reshape(1, F)

    in_maps = []
    for c in range(NC_COUNT):
        sl = slice(c * PB, (c + 1) * PB)
        in_maps.append({
            "node": np.ascontiguousarray(inputs["node"][sl], dtype=np.float32),
            "adj": np.ascontiguousarray(inputs["adj"][sl], dtype=np.float32),
            "dist": np.ascontiguousarray(inputs["dist"][sl], dtype=np.float32),
            "wqkv": wqkv_p,
            "wout": wout,
            "wouth": wouth,
            "bout": bout,
        })
    res = run_bass_kernel_spmd(nc, in_maps, core_ids=list(range(NC_COUNT)),
                               trace=trace)
    out = np.concatenate([res.results[c]["out"] for c in range(NC_COUNT)], axis=0)
    return out, res


def kernel(node, adj, dist, node_mask, adj_mask, dist_mask, W_qkv, W_out, b_out):
    inputs = {"node": np.asarray(node), "adj": np.asarray(adj),
              "dist": np.asarray(dist), "W_qkv": np.asarray(W_qkv),
              "W_out": np.asarray(W_out), "b_out": np.asarray(b_out)}
    out, _ = run(inputs, trace=False)
    return out


# revision 15
# speedup vs baseline: 1.4056x; 1.0843x over previous
"""Trainium2 Bass kernel for nn_Attention_21303037788751 (sparse_attention).

Reference computation (B=16, N=512, F=256, H=8, D=64):
    qkv  = node @ W_qkv                      -> q, k, v  [B,H,N,D]
    attn = softmax(q k^T / sqrt(D)) + 0.5*adj + 0.5*exp(-dist)
    out  = (attn @ v) reshaped  @ W_out + b_out

Sharding: data-parallel over batch, 2 batches per core on 8 NeuronCores.

Per-core program (col-major formulation; all matmuls float32r):
    nodeT = node^T (PE transpose)            [F, N]
    qT,kT per head-pair = W_qk^T @ nodeT     [128=(2 heads), N]
    v     = node @ W_v (row-major)           [N, H*D]
    ST_h  = kT_h-slices ^T @ qT_h            [N_j, N_i]  (K=64, head pairs on
                                              disjoint PE row strips)
    E_h   = exp(ST_h / 8)  (ACT, PSUM->SBUF) -- logits are tiny, no max pass
    OT1_h = vaug_h^T @ E_h: the stationary operand is V_h augmented with a
            ones column (odd heads also get 63 leading zero columns), so one
            accumulation group yields both V^T E and the softmax column sums,
            with odd heads landing at partitions 64..127.
    softmax part = OT1_h * bcast(1/sums)      (DVE recip + DMA bcast)
    G     = adj + exp(-dist); GT by PE transpose
    OT2_p = [v_e|v_o]^T @ GT                  [128, N_i]
    Y     = softmaxT^T @ W_out + OT2^T @ (0.5*W_out) + ones^T @ b_out
"""

import sys

sys.path.insert(0, "/opt/trn_rl_repo")

import numpy as np

B, N, F = 16, 512, 256
H, D = 8, 64
INNER = H * D          # 512
NC_COUNT = 8
PB = B // NC_COUNT     # batches per core
P = 128
SCALE = D ** -0.5      # 0.125
VBLK = 193             # per-pair vaug block: [1 | 0*63 | v_o(64) | v_e(64) | 1]

_CACHE = {}


def _col_perm():
    """Column permutation of W_qkv: head-pair [q_h0|q_h1|k_h0|k_h1] blocks,
    then all v columns grouped by head."""
    order = []
    for p in range(H // 2):
        h0, h1 = 2 * p, 2 * p + 1
        order += [h0 * 192 + d for d in range(64)]
        order += [h1 * 192 + d for d in range(64)]
        order += [h0 * 192 + 64 + d for d in range(64)]
        order += [h1 * 192 + 64 + d for d in range(64)]
    for h in range(H):
        order += [h * 192 + 128 + d for d in range(64)]
    return np.array(order)


def build_program():
    import concourse.bass as bass
    import concourse.tile as tile
    from concourse import bacc, mybir
    from concourse.masks import make_identity

    f32 = mybir.dt.float32
    f32r = mybir.dt.float32r

    nc = bacc.Bacc("TRN2", target_bir_lowering=False, debug=False,
                   num_devices=NC_COUNT)

    node_d = nc.dram_tensor("node", [PB, N, F], f32, kind="ExternalInput").ap()
    adj_d = nc.dram_tensor("adj", [PB, N, N], f32, kind="ExternalInput").ap()
    dist_d = nc.dram_tensor("dist", [PB, N, N], f32, kind="ExternalInput").ap()
    wqkv_d = nc.dram_tensor("wqkv", [F, 3 * INNER], f32, kind="ExternalInput").ap()
    wout_d = nc.dram_tensor("wout", [INNER, F], f32, kind="ExternalInput").ap()
    wouth_d = nc.dram_tensor("wouth", [INNER, F], f32, kind="ExternalInput").ap()
    bout_d = nc.dram_tensor("bout", [1, F], f32, kind="ExternalInput").ap()
    out_d = nc.dram_tensor("out", [PB, N, F], f32, kind="ExternalOutput").ap()

    with tile.TileContext(nc) as tc:
        with tc.tile_pool(name="const", bufs=1) as cpool, \
             tc.tile_pool(name="stage", bufs=1) as spool, \
             tc.tile_pool(name="work1", bufs=1) as wpool1, \
             tc.tile_pool(name="work2", bufs=2) as wpool, \
             tc.tile_pool(name="qk", bufs=4) as qkpool, \
             tc.tile_pool(name="ps_st", bufs=2, space="PSUM") as ps_st, \
             tc.tile_pool(name="ps_ot1", bufs=2, space="PSUM") as ps_ot1, \
             tc.tile_pool(name="ps_misc", bufs=2, space="PSUM") as ps_misc:

            # ---- constants -------------------------------------------------
            ident = cpool.tile([P, P], f32)
            make_identity(nc, ident)

            wqkv_st = spool.tile([P, 2, 3 * INNER], f32, tag="stg")
            nc.sync.dma_start(wqkv_st[:], wqkv_d.rearrange("(kt p) m -> p kt m", p=P))
            wqkv_r = cpool.tile([P, 2, 3 * INNER], f32r)
            nc.vector.tensor_copy(wqkv_r[:], wqkv_st[:])

            wout_st = spool.tile([P, 4, F], f32, tag="stg")
            nc.sync.dma_start(wout_st[:], wout_d.rearrange("(kt p) f -> p kt f", p=P))
            wout_r = cpool.tile([P, 4, F], f32r)
            nc.vector.tensor_copy(wout_r[:], wout_st[:])
            wouth_st = spool.tile([P, 4, F], f32, tag="stg")
            nc.sync.dma_start(wouth_st[:], wouth_d.rearrange("(kt p) f -> p kt f", p=P))
            wouth_r = cpool.tile([P, 4, F], f32r)
            nc.vector.tensor_copy(wouth_r[:], wouth_st[:])

            bout_st = spool.tile([1, F], f32, tag="stg2")
            nc.sync.dma_start(bout_st[:], bout_d[:])
            bout_r = cpool.tile([1, F], f32r)
            nc.vector.tensor_copy(bout_r[:], bout_st[:])

            ones_st = cpool.tile([1, P], f32)
            nc.vector.memset(ones_st[:], 1.0)
            ones_row = cpool.tile([1, P], f32r)
            nc.vector.tensor_copy(ones_row[:], ones_st[:])

            # vaug pad pattern: block head [1, 0*63] (odd-head ones + zeros),
            # block tail col 192 is the even-head ones column
            pat_st = cpool.tile([P, 64], f32)
            nc.vector.memset(pat_st[:], 0.0)
            nc.vector.memset(pat_st[:, 0:1], 1.0)
            one_st = cpool.tile([P, 1], f32)
            nc.vector.memset(one_st[:], 1.0)

            staged = []
            for b in range(PB):
                node_sb = wpool.tile([P, 4, F], f32, tag="node")
                nc.sync.dma_start(node_sb[:],
                                  node_d[b].rearrange("(nb p) f -> p nb f", p=P))
                adj_sb = wpool.tile([P, 4, N], f32, tag="adj")
                nc.sync.dma_start(adj_sb[:],
                                  adj_d[b].rearrange("(ib p) j -> p ib j", p=P))
                dist_sb = wpool.tile([P, 4, N], f32, tag="dist")
                nc.sync.dma_start(dist_sb[:],
                                  dist_d[b].rearrange("(ib p) j -> p ib j", p=P))
                staged.append((node_sb, adj_sb, dist_sb))

            for b in range(PB):
                node_sb, adj_sb, dist_sb = staged[b]
                # ---- nodeT [F, N] ----------------------------------------
                nodeT_r = wpool1.tile([P, 2, N], f32r, tag="nodeT")
                for kt in range(2):
                    tr_ps = ps_misc.tile([P, N], f32, tag="misc")
                    for nb in range(4):
                        nc.tensor.transpose(
                            tr_ps[:, nb * P:(nb + 1) * P],
                            node_sb[:, nb, kt * P:(kt + 1) * P],
                            ident[:])
                    nc.vector.tensor_copy(nodeT_r[:, kt, :], tr_ps[:])

                # ---- v projection into augmented stationary layout --------
                vaug = wpool1.tile([P, 4, 4, VBLK], f32r, tag="v")
                nc.scalar.copy(
                    vaug[:, :, :, 0:64],
                    pat_st[:, None, None, :].to_broadcast((P, 4, 4, 64)))
                nc.scalar.copy(
                    vaug[:, :, :, 192:193],
                    one_st[:, None, None, :].to_broadcast((P, 4, 4, 1)))
                for jb in range(4):
                    v_ps = ps_misc.tile([P, N], f32, tag="misc")
                    for kt in range(2):
                        nc.tensor.matmul(
                            v_ps[:], nodeT_r[:, kt, jb * P:(jb + 1) * P],
                            wqkv_r[:, kt, 2 * INNER:3 * INNER],
                            start=(kt == 0), stop=(kt == 1))
                    v4 = v_ps[:].rearrange("q (pr two d) -> q pr two d",
                                           two=2, d=64)
                    nc.scalar.copy(vaug[:, jb, :, 128:192], v4[:, :, 0, :])
                    nc.scalar.copy(vaug[:, jb, :, 64:128], v4[:, :, 1, :])

                # ---- qT/kT projections ------------------------------------
                qq_r, kk_r = [], []
                for p in range(H // 2):
                    qq_ps = ps_misc.tile([P, N], f32, tag="misc")
                    kk_ps = ps_misc.tile([P, N], f32, tag="misc")
                    base = p * 256
                    for kt in range(2):
                        nc.tensor.matmul(
                            qq_ps[:], wqkv_r[:, kt, base:base + P],
                            nodeT_r[:, kt, :],
                            start=(kt == 0), stop=(kt == 1))
                    for kt in range(2):
                        nc.tensor.matmul(
                            kk_ps[:], wqkv_r[:, kt, base + P:base + 256],
                            nodeT_r[:, kt, :],
                            start=(kt == 0), stop=(kt == 1))
                    qq = qkpool.tile([P, N], f32r, tag="qq")
                    kk = qkpool.tile([P, N], f32r, tag="kk")
                    nc.vector.tensor_copy(qq[:], qq_ps[:])
                    nc.vector.tensor_copy(kk[:], kk_ps[:])
                    qq_r.append(qq)
                    kk_r.append(kk)

                # ---- G = adj + exp(-dist); GT -----------------------------
                nc.scalar.activation(dist_sb[:], dist_sb[:],
                                     mybir.ActivationFunctionType.Exp,
                                     scale=-1.0)
                g_sb = adj_sb
                nc.gpsimd.tensor_add(g_sb[:], adj_sb[:], dist_sb[:])
                gt_r = wpool.tile([P, 4, N], f32r, tag="gt")
                for jb in range(4):
                    tr_ps = ps_misc.tile([P, N], f32, tag="misc")
                    for ib in range(4):
                        nc.tensor.transpose(
                            tr_ps[:, ib * P:(ib + 1) * P],
                            g_sb[:, ib, jb * P:(jb + 1) * P],
                            ident[:])
                    nc.vector.tensor_copy(gt_r[:, jb, :], tr_ps[:])

                # ---- attention per head -----------------------------------
                otfin_r = wpool.tile([P, 4, N], f32r, tag="otfin")
                ot2_r = wpool.tile([P, 4, N], f32r, tag="ot2")
                for h in range(H):
                    p, odd = h // 2, h % 2
                    lo = odd * 64              # row strip within the pair
                    expst = wpool.tile([P, 4, N], f32r, tag="expst")
                    for half in range(2):
                        st_ps = ps_st.tile([P, 2, N], f32, tag="st")
                        for j in range(2):
                            jb = half * 2 + j
                            nc.tensor.matmul(
                                st_ps[:, j, :],
                                kk_r[p][lo:lo + 64, jb * P:(jb + 1) * P],
                                qq_r[p][lo:lo + 64, :],
                                start=True, stop=True)
                        nc.scalar.activation(
                            expst[:, half * 2:half * 2 + 2, :], st_ps[:],
                            mybir.ActivationFunctionType.Exp, scale=SCALE)

                    # V^T E and softmax column sums in one accumulation group
                    ot1_ps = ps_ot1.tile([P, N], f32, tag="ot1")
                    if not odd:
                        out_sl, av_sl, sm_sl = slice(0, 65), slice(0, 64), slice(64, 65)
                    else:
                        out_sl, av_sl, sm_sl = slice(0, P), slice(64, P), slice(0, 1)
                    for jb in range(4):
                        if not odd:
                            lhsT = vaug[:, jb, p, 128:VBLK]
                        else:
                            lhsT = vaug[:, jb, p, 0:128]
                        nc.tensor.matmul(
                            ot1_ps[out_sl, :], lhsT, expst[:, jb, :],
                            start=(jb == 0), stop=(jb == 3))

                    rec = wpool.tile([P, N], f32, tag="rec")
                    if odd:
                        # custom-DVE approx recip is broken at base partition 64
                        nc.vector.reciprocal_approx_fast(rec[sm_sl, :],
                                                         ot1_ps[sm_sl, :])
                    else:
                        nc.vector.reciprocal(rec[sm_sl, :], ot1_ps[sm_sl, :])
                    recbc = wpool.tile([P, N], f32, tag="recbc")
                    nc.sync.dma_start(
                        recbc[av_sl, :],
                        rec[sm_sl, None, :].to_broadcast((1, 64, N)))
                    nc.vector.tensor_tensor(
                        otfin_r[av_sl, p, :], ot1_ps[av_sl, :], recbc[av_sl, :],
                        mybir.AluOpType.mult)

                # ---- OT2 per pair (G part) --------------------------------
                for p in range(H // 2):
                    ot2_ps = ps_misc.tile([P, N], f32, tag="misc")
                    for jb in range(4):
                        nc.tensor.matmul(
                            ot2_ps[:], vaug[:, jb, p, 64:192], gt_r[:, jb, :],
                            start=(jb == 0), stop=(jb == 3))
                    nc.scalar.copy(ot2_r[:, p, :], ot2_ps[:])

                # ---- output projection ------------------------------------
                for nb in range(4):
                    y_ps = ps_misc.tile([P, F], f32, tag="misc")
                    for kt in range(4):
                        nc.tensor.matmul(
                            y_ps[:], otfin_r[:, kt, nb * P:(nb + 1) * P],
                            wout_r[:, kt, :],
                            start=(kt == 0), stop=False)
                    for kt in range(4):
                        nc.tensor.matmul(
                            y_ps[:], ot2_r[:, kt, nb * P:(nb + 1) * P],
                            wouth_r[:, kt, :],
                            start=False, stop=False)
                    nc.tensor.matmul(y_ps[:], ones_row[:], bout_r[:],
                                     start=False, stop=True)
                    y_sb = qkpool.tile([P, F], f32, tag="y")
                    nc.vector.tensor_copy(y_sb[:], y_ps[:])
                    nc.sync.dma_start(out_d[b, nb * P:(nb + 1) * P, :], y_sb[:])

    nc.compile()
    return nc


def _get_program():
    if "nc" not in _CACHE:
        _CACHE["nc"] = build_program()
    return _CACHE["nc"]


def run(inputs, trace=False):
    """Run on 8 cores; returns (full_output, BassKernelResults)."""
    from concourse.bass_utils import run_bass_kernel_spmd

    nc = _get_program()
    wqkv_p = np.ascontiguousarray(inputs["W_qkv"][:, _col_perm()], dtype=np.float32)
    wout = np.ascontiguousarray(inputs["W_out"], dtype=np.float32)
    swap = np.arange(INNER).reshape(4, 2, 64)[:, ::-1, :].reshape(-1)
    wouth = np.ascontiguousarray(0.5 * wout[swap], dtype=np.float32)
    bout = np.ascontiguousarray(inputs["b_out"], dtype=np.float32).reshape(1, F)

    in_maps = []
    for c in range(NC_COUNT):
        sl = slice(c * PB, (c + 1) * PB)
        in_maps.append({
            "node": np.ascontiguousarray(inputs["node"][sl], dtype=np.float32),
            "adj": np.ascontiguousarray(inputs["adj"][sl], dtype=np.float32),
            "dist": np.ascontiguousarray(inputs["dist"][sl], dtype=np.float32),
            "wqkv": wqkv_p,
            "wout": wout,
            "wouth": wouth,
            "bout": bout,
        })
    res = run_bass_kernel_spmd(nc, in_maps, core_ids=list(range(NC_COUNT)),
                               trace=trace)
    out = np.concatenate([res.results[c]["out"] for c in range(NC_COUNT)], axis=0)
    return out, res


def kernel(node, adj, dist, node_mask, adj_mask, dist_mask, W_qkv, W_out, b_out):
    inputs = {"node": np.asarray(node), "adj": np.asarray(adj),
              "dist": np.asarray(dist), "W_qkv": np.asarray(W_qkv),
              "W_out": np.asarray(W_out), "b_out": np.asarray(b_out)}
    out, _ = run(inputs, trace=False)
    return out
